# revision 46
# baseline (speedup 1.0000x reference)
"""Trainium2 Bass kernel for LocalSparseAttention (anti-local windowed attention).

Reference computation (B=2, L=2048, D=512, H=8, hd=64):
    qkv = x @ in_proj_w.T + in_proj_b ; q,k,v = split(qkv)
    q *= 1/sqrt(hd)
    scores = q @ k.T  per head, with positions j in [i-w/2, i+w/2) BANNED (-inf)
    attn = softmax(scores); ctx = attn @ v
    out = LayerNorm(x + ctx @ out_proj_w.T + out_proj_b) * gamma + beta

Sharding: 8 cores = 2 batches x 4 query-shards of 512 rows. Each core
computes k/v for all 2048 keys of its batch (from a host-rotated x^T so
the banned diagonal band lands at fixed key-tile loop positions on every
core; masks are per-core additive {0,-500} strips applied to the raw
scores in PSUM before exp), and full attention + out_proj + residual +
LayerNorm for its 512 queries.

Numerics / speed strategy (validated vs reference in a host numpy sim):
  - scores path (q,k) stays bf16: x^T bf16 + Wq/Wk bf16 prep, q/k casts
    bf16. The 1/sqrt(hd) scale and a softmax-invariant -4 offset are
    folded into the exp (scale=0.125, bias=-4), so exp outputs fit fp8.
  - v path is fp8 with DoubleRow matmuls: x^T fp8 x Wv fp8, contraction
    pairs of 128-chunks per MM -> half the v-prep matmuls. v tiles are
    j-PAIR packed [128, 2, 8*(hd+1)] fp8 with a ones column per head
    (softmax denominator rides the ctx matmul).
  - exp is split across two engines: most units on ACT (Exp activation,
    fp8 out), a few (jp,p2) groups on DVE via the int16 bit-trick
    (y=int16(s*23.083+C) viewed as bf16 ~= exp(s/8-4), +-3.5%, diluted
    ~50x by softmax averaging downstream).
  - ctx = e @ v accumulates over j-PAIRS with fp8 DoubleRow matmuls
    (2 j-tiles per MM); DVE-exp'd pairs use plain bf16-view matmuls.
  - out_proj bf16 (16 MMs) + residual via identity-matrix matmul on a
    bf16 x_nat; LayerNorm in fp32: stats for two query-tiles on DVE
    (bn_stats) and two on ACT (Square/Copy with accum_out row sums),
    rstd via batched DVE reciprocal + ACT Sqrt (table preloaded during
    out_proj), normalize split ACT/DVE.
  - inputs arrive over 3 DGE queues (sync: x^T by segment; scalar: Wq,
    masks; gpsimd SWDGE: bq, Wk, Wv) ordered by first use; fp8 x^T is
    derived on-chip by DVE casts; tail-only tensors (Wout, x_nat, eye)
    DMA mid-kernel; output tiles leave on rotating queues.
"""

import ml_dtypes
import numpy as np

import concourse.bass as bass
import concourse.tile as tile
import concourse.mybir as mybir
from concourse import bacc
from concourse.bass_utils import run_bass_kernel_spmd

F32 = mybir.dt.float32
BF16 = mybir.dt.bfloat16
FP8 = mybir.dt.float8e4
I16 = mybir.dt.int16
AF = mybir.ActivationFunctionType
OP = mybir.AluOpType
DR = mybir.MatmulPerfMode.DoubleRow

B, L, D = 2, 2048, 512
H, HD = 8, 64
SH = L // 4            # 512-query shard per core
NJ = 16                # key tiles of 128 per sequence
UNITS = 2 * NJ         # (j, head-pair) units per head-group
MASK_SLOTS = [0, 1, 2, 3, 4, 15]   # key-tile loop positions that can carry the band
LN_EPS = 1e-5

EXP_SCALE = 0.125          # 1/sqrt(hd)
EXP_OFF = 4.0              # softmax-invariant offset so exp fits fp8
EXP_A = 128.0 / np.log(2.0)
DVE_S1 = float(EXP_A * EXP_SCALE)                    # 23.083
DVE_S2 = float(127 * 128 - 5.0 - EXP_OFF * EXP_A)    # int16 bias
MASK_VAL = -500.0          # additive pre-exp mask (: -62.5 after scale)

# (jp, p2) ctx-pair groups whose two units use the DVE int16 exp instead of
# ACT. Chosen among unmasked j-pairs; tuned for ACT/DVE balance.
DVE_GROUPS = {
    0: {(3, 0), (5, 1), (6, 0)},
    1: {(2, 0), (3, 1), (4, 0), (5, 1), (6, 0)},
}

_COMPILED = None
LAST_RESULT = None
STRIPS = []
LN_TRIVIAL = False


def _pop_schedule(n_items, n_units, lead):
    """Bresenham spread of n_items pops over n_units loop iterations, with a
    `lead`-unit head start so deadlines near the end are met."""
    pops = []
    prev = 0
    for u in range(n_units):
        cur = min(n_items, (u + 1 + lead) * n_items // n_units)
        if u == n_units - 1:
            cur = n_items
        pops.append(cur - prev)
        prev = cur
    return pops


def _strips(half):
    out = []
    for j in MASK_SLOTS[:-1]:
        c0 = max(0, 128 * j - half + 1)
        c1 = min(SH, 128 * j + 128 + half)
        out.append((c0, max(c1, c0 + 1)))
    out.append((0, max(1, min(SH, half))))
    return out


def _build(half, ln_trivial):
    global LN_TRIVIAL, STRIPS
    LN_TRIVIAL = ln_trivial
    STRIPS = _strips(half)
    widths = [c1 - c0 for c0, c1 in STRIPS]
    offs = np.concatenate([[0], np.cumsum(widths)]).tolist()

    nc = bacc.Bacc("TRN2", target_bir_lowering=False, debug=False, num_devices=8)

    xT = nc.dram_tensor("xT", [D, L], BF16, kind="ExternalInput")          # rotated x^T bf16
    wqkT = nc.dram_tensor("wqkT", [D, 2 * D], BF16, kind="ExternalInput")  # in_proj_w.T q,k cols
    wv8T = nc.dram_tensor("wv8T", [D, D], FP8, kind="ExternalInput")       # in_proj_w.T v cols fp8
    woutT = nc.dram_tensor("woutT", [D, D], BF16, kind="ExternalInput")    # out_proj_w.T
    x_nat = nc.dram_tensor("x_nat", [SH, D], BF16, kind="ExternalInput")   # query shard rows (+b_out folded)
    bq_d = nc.dram_tensor("bq", [128, 4], F32, kind="ExternalInput")       # q bias, chunked (unscaled)
    gamma_d = nc.dram_tensor("gamma", [128, D], F32, kind="ExternalInput")
    beta_d = nc.dram_tensor("beta", [128, D], F32, kind="ExternalInput")
    masks_d = nc.dram_tensor("masks", [128, offs[-1]], BF16, kind="ExternalInput")  # additive strips
    eye_d = nc.dram_tensor("eye", [128, 128], BF16, kind="ExternalInput")  # identity (residual matmul)
    out_d = nc.dram_tensor("out", [SH, D], F32, kind="ExternalOutput")

    xT_v = xT.rearrange("(d p) c -> p d c", p=128)          # [128, 4, L]
    wqk_v = wqkT.rearrange("(d p) c -> p d c", p=128)       # [128, 4, 2D]
    wv8_v = wv8T.rearrange("(d p) c -> p d c", p=128)       # [128, 4, D]
    wout_v = woutT.rearrange("(d p) c -> p d c", p=128)     # [128, 4, D]
    x_nat_v = x_nat.rearrange("(t p) c -> p t c", p=128)    # [128, 4, D]

    with tile.TileContext(nc) as tc:
        with (
            tc.tile_pool(name="persist", bufs=1) as pp,
            tc.tile_pool(name="work", bufs=2) as wp,
            tc.tile_pool(name="kvsb", bufs=1) as kvsb,
            tc.tile_pool(name="ep8", bufs=2) as ep8,
            tc.tile_pool(name="ep16", bufs=2) as ep16,
        ):
            # ---- input DMAs over 4 DGE queues, in first-use order ----
            # scalar queue: q/k weights, then out-proj weight
            wqk_sb = pp.tile([128, 4 * 2 * D], BF16, tag="wqk")
            wqk_r = wqk_sb.rearrange("p (d c) -> p d c", d=4)
            nc.scalar.dma_start(out=wqk_r[:, :, 0:D], in_=wqk_v[:, :, 0:D])
            # sync queue: x^T bf16 segments 0-1; segments 2-3 ride the
            # gpsimd queue (issued there after wk/wv8) so all four land
            # ~4us earlier than a single-queue stream
            xbf_sb = pp.tile([128, 4 * L], BF16, tag="xbf")
            xbr = xbf_sb.rearrange("p (d c) -> p d c", d=4)
            for seg in range(2):
                nc.sync.dma_start(out=xbr[:, :, 512 * seg:512 * seg + 512],
                                  in_=xT_v[:, :, 512 * seg:512 * seg + 512])
            # gpsimd queue: bq, k weights, v weights fp8 (tail tensors deferred)
            bq_sb = pp.tile([128, 4], F32, tag="bq")
            nc.gpsimd.dma_start(out=bq_sb, in_=bq_d[:, :])
            nc.gpsimd.dma_start(out=wqk_r[:, :, D:2 * D], in_=wqk_v[:, :, D:2 * D])
            wv8_sb = pp.tile([128, 4 * D], FP8, tag="wv8")
            nc.gpsimd.dma_start(
                out=wv8_sb.rearrange("p (d c) -> p d c", d=4), in_=wv8_v[:, :, :],
            )
            for seg in (2, 3):
                nc.gpsimd.dma_start(out=xbr[:, :, 512 * seg:512 * seg + 512],
                                    in_=xT_v[:, :, 512 * seg:512 * seg + 512])
            # scalar queue: masks after wq
            mask_sb = pp.tile([128, offs[-1]], BF16, tag="masks")
            nc.scalar.dma_start(out=mask_sb, in_=masks_d[:, :])
            # x^T fp8 for the v path is derived on-chip (DVE casts per segment)
            x8_sb = pp.tile([128, 4 * L], FP8, tag="x8")
            x8r = x8_sb.rearrange("p (d c) -> p d c", d=4)
            # tail-only tiles; their DMAs are issued mid-kernel
            wout_sb = pp.tile([128, 4 * D], BF16, tag="wout")
            x_nat_sb = pp.tile([128, 4 * D], BF16, tag="xnat")
            eye_sb = pp.tile([128, 128], BF16, tag="eye")
            gamma_sb = beta_sb = None
            if not LN_TRIVIAL:
                gamma_sb = pp.tile([128, D], F32, tag="gamma")
                beta_sb = pp.tile([128, D], F32, tag="beta")

            def emit_x8cast(seg):
                nc.vector.tensor_copy(
                    x8r[:, :, 512 * seg:512 * seg + 512],
                    xbr[:, :, 512 * seg:512 * seg + 512],
                )

            def emit_deferred_dmas():
                nc.scalar.dma_start(
                    out=wout_sb.rearrange("p (d c) -> p d c", d=4), in_=wout_v[:, :, :],
                )
                nc.gpsimd.dma_start(
                    out=x_nat_sb.rearrange("p (t c) -> p t c", t=4), in_=x_nat_v[:, :, :],
                )
                nc.gpsimd.dma_start(out=eye_sb, in_=eye_d[:, :])
                if not LN_TRIVIAL:
                    nc.gpsimd.dma_start(out=gamma_sb, in_=gamma_d[:, :])
                    nc.gpsimd.dma_start(out=beta_sb, in_=beta_d[:, :])

            def xc(d):          # bf16 xT d-chunk view [128, L]
                return xbf_sb[:, L * d:L * (d + 1)]

            # ---- constants ----
            wup = pp.tile([128, 128], BF16, tag="wup")
            nc.vector.memset(wup, 0.001)
            eps_t = pp.tile([128, 1], F32, tag="eps")
            nc.vector.memset(eps_t, LN_EPS)
            expb_t = pp.tile([128, 1], F32, tag="expb")
            nc.vector.memset(expb_t, -EXP_OFF)
            exps_t = pp.tile([128, 1], F32, tag="exps")
            nc.vector.memset(exps_t, EXP_SCALE)
            ctxTs_sb = [pp.tile([128, SH], BF16, tag=f"ctxTs{p}", name=f"ctxTs{p}") for p in range(4)]
            ctxraw_sb = [pp.tile([65, SH], F32, tag=f"ctxraw{t}", name=f"ctxraw{t}") for t in range(4)]
            qT_sb = [pp.tile([128, SH], BF16, tag=f"qT{c2}", name=f"qT{c2}") for c2 in range(4)]

            # v j-pair tiles [128, 2, 8, 80] fp8 (per-head stride padded to 80
            # bytes: DoubleRow LDWEIGHTS requires 16B-aligned steps); col 64 of
            # each head is the ones column (softmax denominator), memset once
            VWP = 80
            VW = H * VWP
            v_sb = [kvsb.tile([128, 2 * VW], FP8, tag=f"v{jp}", name=f"v{jp}") for jp in range(NJ // 2)]
            for jp in range(NJ // 2):
                vv = v_sb[jp].rearrange("p (s h c) -> p s h c", s=2, c=VWP)
                nc.vector.memset(vv[:, :, :, HD:HD + 1], 1.0)
            kt_sb = [kvsb.tile([128, L], BF16, tag=f"kt{c2}", name=f"kt{c2}") for c2 in range(4)]

            def emit_q(c, pool, tag, bufs=2):
                ps = pool.tile([128, SH], F32, tag=tag, bufs=bufs, name=f"qps{c}")
                for d in range(4):
                    nc.tensor.matmul(
                        ps,
                        wqk_sb[:, 2 * D * d + 128 * c:2 * D * d + 128 * c + 128],
                        xc(d)[:, 0:SH],
                        start=(d == 0), stop=(d == 3),
                    )
                nc.vector.tensor_scalar_add(qT_sb[c], ps, bq_sb[:, c:c + 1])

            def keep_warm(moving, pool, tag="wu"):
                dps = pool.tile([128, 128], F32, tag=tag, name="kw")
                nc.tensor.matmul(
                    dps, wup, moving, start=True, stop=True,
                    skip_group_check=True,
                )

            # PE warm-up burst + q projection for group 0
            with tc.tile_pool(name="wups", bufs=1, space="PSUM") as wps:
                wq_ps = wps.tile([128, 512], F32, tag="wux")
                for i in range(48):
                    nc.tensor.matmul(
                        wq_ps[:, 0:128], wup, wup,
                        start=(i == 0), stop=(i == 47),
                    )
                with tc.tile_pool(name="qps", bufs=2, space="PSUM") as qps:
                    for d in range(4):
                        keep_warm(xc(d)[:, 0:128], wps)
                    emit_q(0, qps, "q")
                    keep_warm(wqk_sb[:, 0:128], wps)
                    emit_q(1, qps, "q")
                    keep_warm(wv8_sb[:, 0:128], wps)
                    keep_warm(mask_sb[:, 0:128], wps)

            def emit_kt(c2, seg, pool, tag="sc", on_act=False):
                """k^T chunk c2, one 512-key segment: 4 bf16 MMs + bf16 cast."""
                ps = pool.tile([128, 512], F32, tag=tag, bufs=3, name=f"ktps{c2}_{seg}")
                for d in range(4):
                    nc.tensor.matmul(
                        ps,
                        wqk_sb[:, 2 * D * d + D + 128 * c2:2 * D * d + D + 128 * c2 + 128],
                        xc(d)[:, 512 * seg:512 * seg + 512],
                        start=(d == 0), stop=(d == 3),
                    )
                if on_act:
                    nc.scalar.activation(kt_sb[c2][:, 512 * seg:512 * seg + 512], ps, AF.Copy)
                else:
                    nc.vector.tensor_copy(kt_sb[c2][:, 512 * seg:512 * seg + 512], ps)

            def emit_v(l2, pool, tag="sc", on_act=False):
                """v for key tile l2: 2 fp8 DoubleRow MMs + fp8 strided cast."""
                ps = pool.tile([128, 512], F32, tag=tag, bufs=3, name=f"vps{l2}")
                x8vv = x8r[:, :, 128 * l2:128 * l2 + 128]      # [128, 4, 128]
                w8vv = wv8_sb.rearrange("p (d c) -> p d c", d=4)
                nc.tensor.matmul(
                    ps, x8vv[:, 0:2, :], w8vv[:, 0:2, :],
                    start=True, stop=False, perf_mode=DR,
                )
                nc.tensor.matmul(
                    ps, x8vv[:, 2:4, :], w8vv[:, 2:4, :],
                    start=False, stop=True, perf_mode=DR,
                )
                vv = v_sb[l2 // 2].rearrange("p (s h c) -> p s h c", s=2, c=VWP)
                if on_act:
                    nc.scalar.activation(
                        vv[:, l2 % 2, :, 0:HD],
                        ps.rearrange("p (t c) -> p t c", c=HD), AF.Copy,
                    )
                else:
                    nc.vector.tensor_copy(
                        vv[:, l2 % 2, :, 0:HD],
                        ps.rearrange("p (t c) -> p t c", c=HD),
                    )

            # prep emissions are popped in PAIRS so the number of "sc"-tag
            # PSUM allocations between consecutive scores stays even -- this
            # keeps the two scores slots alternating (a lone prep allocation
            # would give every scores tile the SAME slot, serializing
            # scores(u+1) behind exp(u)). x8 casts are DVE-only (no psum
            # allocation) and are emitted separately.
            queue_g0 = [
                ("v", 2, None), ("v", 3, None), ("q", 2, None), ("q", 3, None),
                ("kt", 0, 1), ("kt", 1, 1), ("x8c", 1, None), ("v", 4, None),
                ("v", 5, None), ("v", 6, None), ("v", 7, None),
                ("kt", 0, 2), ("kt", 1, 2), ("x8c", 2, None), ("v", 8, None),
                ("v", 9, None), ("v", 10, None), ("v", 11, None),
                ("kt", 0, 3), ("kt", 1, 3), ("x8c", 3, None), ("v", 12, None),
                ("v", 13, None), ("v", 14, None), ("v", 15, None),
                ("kt", 2, 0), ("kt", 3, 0),
            ]
            pops_g0 = _pop_schedule(len(queue_g0), UNITS, lead=3)
            queue_g1 = [
                ("kt", 2, 1), ("kt", 3, 1),
                ("kt", 2, 2), ("kt", 3, 2),
                ("kt", 2, 3), ("kt", 3, 3),
            ]
            pops_g1 = [0] * UNITS
            for u2 in (2, 4, 10, 12, 18, 20):
                pops_g1[u2] += 1

            def emit_scores(g, u, scp):
                """raw scores for unit u of group g (+ additive mask);
                returns (j, p2, sc_tile)."""
                j, p2 = divmod(u, 2)
                sc = scp.tile([128, 2 * SH], F32, tag="sc", bufs=3,
                              name=f"sc{g}_{u}")
                for t in range(2):
                    nc.tensor.matmul(
                        sc[:, SH * t:SH * t + SH],
                        kt_sb[2 * g + p2][64 * t:64 * t + 64, 128 * j:128 * j + 128],
                        qT_sb[2 * g + p2][64 * t:64 * t + 64, :],
                        start=True, stop=True,
                    )
                if j in MASK_SLOTS:
                    slot = MASK_SLOTS.index(j)
                    c0, c1 = STRIPS[slot]
                    wdt = c1 - c0
                    scv = sc.rearrange("p (t q) -> p t q", t=2)[:, :, c0:c1]
                    mb = mask_sb[:, offs[slot]:offs[slot] + wdt].rearrange(
                        "p (a q) -> p a q", a=1
                    ).broadcast_to((128, 2, wdt))
                    nc.vector.tensor_tensor(out=scv, in0=scv, in1=mb, op=OP.add)
                return (j, p2, sc)

            # per-pass current e-pair tile bookkeeping
            cur_ep = {}

            def emit_exp2(p, g, p2, j, sc, dve):
                """exp for scores of local key-tile j; writes into the pass's
                pair tile slot j%2. Returns (j, dve) consumed by ctx."""
                jp = j // 2
                if j % 2 == 0:
                    if dve:
                        t16 = ep16.tile([128, 2 * 2 * SH], I16, tag=f"ei{p2}", name=f"ei{p}_{jp}")
                        cur_ep[p] = t16
                    else:
                        t8 = ep8.tile([128, 2 * 2 * SH], FP8, tag=f"ep{p2}", name=f"ep{p}_{jp}")
                        cur_ep[p] = t8
                tile_e = cur_ep[p]
                ev = tile_e.rearrange("p (s q) -> p s q", s=2)[:, j % 2, :]
                if dve:
                    nc.vector.tensor_scalar(
                        out=ev, in0=sc, scalar1=DVE_S1, scalar2=DVE_S2,
                        op0=OP.mult, op1=OP.add,
                    )
                else:
                    nc.scalar.activation(ev, sc, AF.Exp, bias=expb_t, scale=exps_t)
                return (j, dve, tile_e)

            def emit_ctx_pair(g, j, p2, dve, ctx_ps, tile_e=None):
                """ctx accumulation for completed j-pair (j-1, j) of head-pair
                p2: fp8 DoubleRow (2 MMs) or bf16-view plain (4 MMs)."""
                jp = j // 2
                vv = v_sb[jp].rearrange("p (s c) -> p s c", s=2)
                if not dve:
                    epv = tile_e.rearrange("p (s q) -> p s q", s=2)
                    for t in range(2):
                        h = 4 * g + 2 * p2 + t
                        nc.tensor.matmul(
                            ctx_ps[2 * p2 + t],
                            vv[:, :, VWP * h:VWP * h + HD + 1],
                            epv[:, :, SH * t:SH * t + SH],
                            start=(jp == 0), stop=(jp == NJ // 2 - 1),
                            perf_mode=DR,
                        )
                else:
                    ebf = tile_e.bitcast(BF16).rearrange("p (s q) -> p s q", s=2)
                    for t in range(2):
                        h = 4 * g + 2 * p2 + t
                        for s in range(2):
                            nc.tensor.matmul(
                                ctx_ps[2 * p2 + t],
                                vv[:, s, VWP * h:VWP * h + HD + 1],
                                ebf[:, s, SH * t:SH * t + SH],
                                start=(jp == 0 and s == 0),
                                stop=(jp == NJ // 2 - 1 and s == 1),
                            )

            def divide_ops(g, src_tiles, act_copies, hts=None, interleave2=False):
                """closure list normalizing the 4 head-pairs (see bf16 version)."""
                ops_list = []
                state = {}

                def mk(ht):
                    p2, t = divmod(ht, 2)
                    src = src_tiles[ht]

                    def op_copy():
                        sA = wp.tile([1, SH], F32, tag=f"sA{ht}", name=f"sA{g}_{ht}")
                        if act_copies:
                            nc.scalar.activation(sA, src[HD:HD + 1, :], AF.Copy)
                        else:
                            nc.vector.tensor_copy(sA, src[HD:HD + 1, :])
                        state[(ht, "sA")] = sA

                    def op_recip():
                        lg = wp.tile([1, SH], F32, tag=f"lg{ht}", name=f"lg{g}_{ht}")
                        nc.vector.reciprocal_approx_fast(lg, state[(ht, "sA")])
                        state[(ht, "lg")] = lg

                    def op_bcast():
                        bc_sb = wp.tile([HD, SH], F32, tag=f"bcsb{ht}", name=f"bcsb{g}_{ht}")
                        nc.gpsimd.partition_broadcast(bc_sb, state[(ht, "lg")])
                        state[(ht, "bc")] = bc_sb

                    def op_mult():
                        nc.vector.tensor_tensor(
                            out=ctxTs_sb[2 * g + p2][64 * t:64 * t + 64, :],
                            in0=src[0:HD, :],
                            in1=state[(ht, "bc")],
                            op=OP.mult,
                        )
                    return [op_copy, op_recip, op_bcast, op_mult]

                for ht in range(4):
                    ops_list.append(mk(ht))
                if hts is not None and interleave2:
                    a, b = hts
                    idx2 = {a: 0, b: 0}
                    flat = []
                    for ht in (a, b, a, a, b, a, b, b):
                        flat.append(ops_list[ht][idx2[ht]])
                        idx2[ht] += 1
                    return flat
                if hts is not None:
                    flat = []
                    for ht in hts:
                        flat.extend(ops_list[ht])
                    return flat
                flat = []
                chains = ops_list
                idx = [0, 0, 0, 0]
                order = [0, 1, 0, 0, 2, 1, 1, 3, 2, 0, 2, 1, 3, 2, 3, 3]
                for ht in order:
                    flat.append(chains[ht][idx[ht]])
                    idx[ht] += 1
                return flat

            # ---- four (group, head-pair) passes of 16 key-tiles each.
            # Serializing the two head-pair streams keeps only 2 ctx banks
            # alive at a time, which frees PSUM for a 3-deep scores rotation
            # (6 banks) -- breaking the exp(u)->scores(u+2)->exp(u+2)
            # 2-slot recurrence that bound the interleaved version. ----
            PASS_POPS = {
                0: {0: [("v", 2, None)], 1: [("v", 3, None)],
                    2: [("v", 4, None)], 3: [("v", 5, None)],
                    4: [("v", 6, None)], 5: [("v", 7, None)],
                    6: [("kt", 0, 2)], 7: [("v", 8, None)],
                    8: [("v", 9, None)], 9: [("kt", 0, 3)],
                    10: [("v", 10, None)], 11: [("v", 11, None)],
                    12: [("v", 12, None), ("kt", 1, 0)],
                    13: [("v", 13, None), ("q", 1, None)],
                    14: [("v", 14, None), ("kt", 1, 1)],
                    15: [("v", 15, None)]},
                1: {2: [("kt", 1, 2)], 6: [("kt", 1, 3)], 10: [("kt", 2, 0)],
                    12: [("q", 2, None)], 14: [("kt", 2, 1)]},
                2: {2: [("kt", 2, 2)], 6: [("kt", 2, 3)], 10: [("kt", 3, 0)],
                    12: [("q", 3, None)], 14: [("kt", 3, 1)]},
                3: {2: [("kt", 3, 2)], 6: [("kt", 3, 3)]},
            }
            PASS_X8C = {0: {1: 1, 5: 2, 9: 3}}
            PASS_DVE = {0: {6}, 1: {2, 5}, 2: {3, 6}, 3: {2, 4, 6}}

            with tc.tile_pool(name="scps", bufs=3, space="PSUM") as scp:
                # pre-loop: q0, kt0 segs 0-1, v tiles 0-5 (gated by x^T segs
                # 0-1 as they stream in)
                emit_x8cast(0)
                emit_q(0, scp, "sc", bufs=3)
                emit_kt(0, 0, scp, on_act=True)
                emit_v(0, scp, on_act=True)
                emit_v(1, scp, on_act=True)
                emit_kt(0, 1, scp, on_act=True)
                hoisted = []
                lazy_div = []
                for p in range(4):
                    g, p2 = divmod(p, 2)
                    pops = PASS_POPS[p]
                    x8c_at = PASS_X8C.get(p, {})
                    with tc.tile_pool(name=f"ctxps{p}", bufs=1, space="PSUM") as cxp:
                        cpair = [cxp.tile([65, SH], F32, tag=f"ctx{t}", name=f"ctx{p}_{t}") for t in range(2)]
                        ctx_ps = [None] * 4
                        ctx_ps[2 * p2] = cpair[0]
                        ctx_ps[2 * p2 + 1] = cpair[1]
                        pend_q = []
                        edone = None
                        for j in range(NJ + 3):
                            if p == 0 and j == 8:
                                emit_deferred_dmas()
                            if lazy_div and j >= 3:
                                lazy_div.pop(0)()
                            if j < NJ:
                                if j in x8c_at:
                                    emit_x8cast(x8c_at[j])
                                for kind, a, b2 in pops.get(j, ()):
                                    if kind == "kt":
                                        emit_kt(a, b2, scp)
                                    elif kind == "q":
                                        emit_q(a, scp, "sc", bufs=3)
                                    else:
                                        emit_v(a, scp)
                            if j < len(hoisted):
                                pend_q.append(hoisted[j])
                            elif j < NJ:
                                pend_q.append(emit_scores(g, 2 * j + p2, scp))
                            if len(pend_q) > 2 or (j >= NJ and pend_q):
                                jj, _, sc = pend_q.pop(0)
                                dve = (jj // 2) in PASS_DVE[p]
                                rec = emit_exp2(p, g, p2, jj, sc, dve)
                                if edone is not None and edone[0] % 2 == 1:
                                    emit_ctx_pair(g, edone[0], p2, edone[1], ctx_ps, tile_e=edone[2])
                                edone = rec
                        if edone is not None and edone[0] % 2 == 1:
                            emit_ctx_pair(g, edone[0], p2, edone[1], ctx_ps, tile_e=edone[2])

                        # dump raw ctx; next pass's first two scores hoisted;
                        # this pass's softmax divides run lazily in the next
                        raw0 = ctxraw_sb[(p % 2) * 2]
                        raw1 = ctxraw_sb[(p % 2) * 2 + 1]
                        nc.vector.tensor_copy(raw0, cpair[0])
                        nc.vector.tensor_copy(raw1, cpair[1])
                        hoisted = []
                        if p < 3:
                            ng, np2 = divmod(p + 1, 2)
                            hoisted = [emit_scores(ng, 0 + np2, scp),
                                       emit_scores(ng, 2 + np2, scp),
                                       emit_scores(ng, 4 + np2, scp),
                                       emit_scores(ng, 6 + np2, scp)]
                            src = [None] * 4
                            src[2 * p2] = raw0
                            src[2 * p2 + 1] = raw1
                            lazy_div = divide_ops(g, src, act_copies=False,
                                                  hts=(2 * p2, 2 * p2 + 1),
                                                  interleave2=True)
                        else:
                            pass

            # ---- out_proj + residual + LayerNorm ----
            with tc.tile_pool(name="ops", bufs=1, space="PSUM") as ops:
                po_t = [ops.tile([128, D], F32, tag=f"po{qt}", name=f"po{qt}") for qt in range(4)]
                for qt in range(4):
                    nc.tensor.matmul(
                        po_t[qt], eye_sb, x_nat_sb[:, D * qt:D * qt + D],
                        start=True, stop=False,
                    )
                for op in divide_ops(1, [None, None, ctxraw_sb[2], ctxraw_sb[3]],
                                     act_copies=True, hts=(2, 3),
                                     interleave2=True):
                    op()
                # HAM keep-alive through the divide wait (chained on divide
                # outputs so they run late, not early)
                keep_warm(ctxTs_sb[2][:, 0:128], ops, tag="kwt0")
                keep_warm(ctxTs_sb[3][:, 0:128], ops, tag="kwt1")
                # preload the sqrt table now (chained on the tail divide's
                # output so the scheduler cannot hoist it before the exps)
                sqd = wp.tile([128, 1], F32, tag="sqd")
                nc.scalar.activation(sqd, ctxTs_sb[3][:, 0:1], AF.Sqrt)
                for p in range(4):
                    for qt in range(4):
                        nc.tensor.matmul(
                            po_t[qt],
                            ctxTs_sb[p][:, 128 * qt:128 * qt + 128],
                            wout_sb[:, D * p:D * p + D],
                            start=False, stop=(p == 3),
                        )
                # LayerNorm: stats for qt 0/1 on DVE (bn_stats), qt 2/3 on ACT
                # (Square/Copy passes with accum_out row sums); one batched
                # reciprocal+Sqrt; normalize qt 0/1 on ACT, qt 2/3 on DVE;
                # output DMAs on rotating queues.
                veps4 = wp.tile([128, 4], F32, tag="veps4")
                negmu4 = wp.tile([128, 4], F32, tag="negmu4")
                sums = wp.tile([128, 4], F32, tag="lnsums")
                for qt in (2,):
                    scr = wp.tile([128, D], F32, tag="lnscr", name=f"scr{qt}")
                    nc.scalar.activation(scr, po_t[qt], AF.Square,
                                         accum_out=sums[:, qt:qt + 1])
                    scr2 = wp.tile([128, D], F32, tag="lnscr", name=f"scr2{qt}")
                    nc.scalar.activation(scr2, po_t[qt], AF.Copy,
                                         accum_out=sums[:, qt - 2:qt - 1])
                for qt in (0, 1, 3):
                    stats = wp.tile([128, 6], F32, tag="stats", name=f"st{qt}")
                    nc.vector.bn_stats(stats, po_t[qt])
                    mv = wp.tile([128, 2], F32, tag="mv", name=f"mv{qt}")
                    nc.vector.bn_aggr(mv, stats)
                    nc.vector.tensor_scalar_add(veps4[:, qt:qt + 1], mv[:, 1:2], eps_t)
                    nc.vector.tensor_scalar(
                        out=negmu4[:, qt:qt + 1], in0=mv[:, 0:1],
                        scalar1=-1.0, scalar2=0.0, op0=OP.mult, op1=OP.add,
                    )
                for qt in (2,):
                    # negmu = -sumy/D ; veps = sumsq/D - mu^2 + eps
                    nc.vector.tensor_scalar(
                        out=negmu4[:, qt:qt + 1], in0=sums[:, qt - 2:qt - 1],
                        scalar1=-1.0 / D, scalar2=0.0, op0=OP.mult, op1=OP.add,
                    )
                    mu2 = wp.tile([128, 1], F32, tag="mu2", name=f"mu2{qt}")
                    nc.vector.tensor_tensor(out=mu2, in0=negmu4[:, qt:qt + 1],
                                            in1=negmu4[:, qt:qt + 1], op=OP.mult)
                    nc.vector.tensor_scalar(
                        out=veps4[:, qt:qt + 1], in0=sums[:, qt:qt + 1],
                        scalar1=1.0 / D, scalar2=eps_t, op0=OP.mult, op1=OP.add,
                    )
                    nc.vector.tensor_tensor(out=veps4[:, qt:qt + 1],
                                            in0=veps4[:, qt:qt + 1],
                                            in1=mu2, op=OP.subtract)
                rec4 = wp.tile([128, 4], F32, tag="rec4")
                nc.vector.reciprocal(rec4, veps4)
                rstd4 = wp.tile([128, 4], F32, tag="rstd4")
                nc.scalar.activation(rstd4, rec4, AF.Sqrt)
                nmb4 = wp.tile([128, 4], F32, tag="nmb4")
                nc.vector.tensor_tensor(out=nmb4, in0=negmu4, in1=rstd4, op=OP.mult)
                dma_engines = [nc.sync, nc.scalar, nc.gpsimd, nc.sync]
                for qt in range(4):
                    t1 = wp.tile([128, D], F32, tag=f"t1{qt}", name=f"t1{qt}")
                    if qt != 2:
                        nc.scalar.activation(
                            t1, po_t[qt], AF.Identity,
                            bias=nmb4[:, qt:qt + 1], scale=rstd4[:, qt:qt + 1],
                        )
                    else:
                        nc.vector.tensor_scalar(
                            out=t1, in0=po_t[qt],
                            scalar1=rstd4[:, qt:qt + 1], scalar2=nmb4[:, qt:qt + 1],
                            op0=OP.mult, op1=OP.add,
                        )
                    if not LN_TRIVIAL:
                        nc.vector.tensor_tensor(out=t1, in0=t1, in1=gamma_sb, op=OP.mult)
                        nc.vector.tensor_tensor(out=t1, in0=t1, in1=beta_sb, op=OP.add)
                    dma_engines[qt].dma_start(out=out_d[128 * qt:128 * qt + 128, :], in_=t1)

    nc.compile()
    return nc


def _host_prep(x, in_proj_w, in_proj_b, out_proj_w, out_proj_b, ln_gamma, ln_beta, window_size):
    x = np.ascontiguousarray(np.asarray(x, dtype=np.float32))
    in_proj_w = np.asarray(in_proj_w, dtype=np.float32)
    in_proj_b = np.asarray(in_proj_b, dtype=np.float32)
    out_proj_w = np.asarray(out_proj_w, dtype=np.float32)
    out_proj_b = np.asarray(out_proj_b, dtype=np.float32)
    ln_gamma = np.asarray(ln_gamma, dtype=np.float32)
    ln_beta = np.asarray(ln_beta, dtype=np.float32)
    w = int(np.asarray(window_size))
    half = w // 2
    assert half <= 128, "mask slots only cover |k-q| <= 128"
    strips = _strips(half)

    bf16 = ml_dtypes.bfloat16
    fp8 = ml_dtypes.float8_e4m3
    W = in_proj_w
    wqkT = np.ascontiguousarray(W[0:2 * D].T.astype(bf16))      # [D, 2D] (q unscaled)
    wv8T = np.ascontiguousarray(W[2 * D:3 * D].T.astype(fp8))   # [D, D]
    woutT = np.ascontiguousarray(out_proj_w.T.astype(bf16))     # [D, D]
    bq = np.ascontiguousarray(in_proj_b[0:D].reshape(4, 128).T)  # [128, 4] unscaled
    bout = (out_proj_b + out_proj_w @ in_proj_b[2 * D:3 * D]).reshape(1, D)
    gamma_b = np.ascontiguousarray(np.broadcast_to(ln_gamma, (128, D)))
    beta_b = np.ascontiguousarray(np.broadcast_to(ln_beta, (128, D)))

    in_maps = []
    for c in range(8):
        b, s = divmod(c, 4)
        rot = (SH * s + np.arange(L)) % L
        xrot = x[b][rot]
        xT_rot = np.ascontiguousarray(xrot.T.astype(bf16))       # [D, L]
        x_nat = np.ascontiguousarray(
            (x[b][SH * s:SH * s + SH] + bout[None, 0, :]).squeeze().astype(bf16))
        q_true = SH * s + np.arange(SH)[None, :]
        mstrips = []
        for i, j in enumerate(MASK_SLOTS):
            c0, c1 = strips[i]
            k_true = (SH * s + 128 * j + np.arange(128)[:, None]) % L
            dd = k_true - q_true[:, :]
            banned = (dd >= -half) & (dd < half)
            mstrips.append((banned[:, c0:c1] * np.float32(MASK_VAL)).astype(bf16))
        masks = np.ascontiguousarray(np.concatenate(mstrips, axis=1))
        in_maps.append({
            "xT": xT_rot, "x_nat": x_nat,
            "wqkT": wqkT, "wv8T": wv8T, "woutT": woutT,
            "bq": bq, "gamma": gamma_b, "beta": beta_b,
            "masks": masks, "eye": np.eye(128, dtype=np.float32).astype(bf16),
        })
    return in_maps


def kernel(x, in_proj_w, in_proj_b, out_proj_w, out_proj_b, ln_gamma, ln_beta, window_size):
    global _COMPILED, LAST_RESULT
    half = int(np.asarray(window_size)) // 2
    ln_trivial = bool(np.all(np.asarray(ln_gamma) == 1.0) and np.all(np.asarray(ln_beta) == 0.0))
    key = (half, ln_trivial)
    if _COMPILED is None or _COMPILED[0] != key:
        _COMPILED = (key, _build(half, ln_trivial))
    in_maps = _host_prep(x, in_proj_w, in_proj_b, out_proj_w, out_proj_b,
                         ln_gamma, ln_beta, window_size)
    res = run_bass_kernel_spmd(_COMPILED[1], in_maps, core_ids=list(range(8)))
    LAST_RESULT = res
    out = np.empty((B, L, D), np.float32)
    for c in range(8):
        b, s = divmod(c, 4)
        out[b, SH * s:SH * s + SH] = res.results[c]["out"]
    return out
